# revision 22
# baseline (speedup 1.0000x reference)
"""Trainium2 Bass kernel for nn_Envelopes (moe_routing).

Math being implemented (per batch b, event e):
  w[e]   = max(softmax(selections[b,e,:])) = 1 / sum(exp(x - max(x)))
  row    = argmax(selections[b,e,:])
  sel    = w[e] * items_norm[row, :]        items_norm = (items - min)/(max-min+1e-3)
  amp    = linear_upsample_16x(sel)         (half-pixel centers, edge clamp)
  out    = concat([amp * noise[b,e,:], zeros(32768)])

Kernel strategy (one core per batch, 8 cores, NO collectives):
  - global table min/max is computed LOCALLY on every core: the full 4 MiB
    table is streamed in as [128, 8192] (4 pipelined 1 MiB chunks), reduced
    per-partition on DVE, then one gpsimd partition_all_reduce(128) gives
    every partition [max, -min].  This removes the cross-core AllReduce +
    DRAM bounce + broadcast of the earlier design: +3.5 MiB of HBM reads
    buys the whole min/max chain off the critical path.
  - argmax row ids via DVE max/max_index; rows fetched with one indirect DMA
  - the affine normalization folds into the gathered rows BEFORE
    interpolation: gsc = g*(w*inv) + (w*(-min)*inv).  Linear interpolation
    with unit column sums commutes with affine maps, so this is exact, and
    the per-output-chunk epilogue collapses to a single DVE multiply
    out = psum * noise.
  - window tiles T[ci] = PE transposes of the scaled rows (66-sample
    overlapping windows; edge clamp = replicated halo columns)
  - 16x upsample = matmul with a constant 66x1024 triangle-filter matrix W
    (each output column holds the 2 taps of linear interpolation)
  - outputs stream out in a [128, 16384] flat layout (partition = (event,
    half)) so every big DMA uses all 128 partitions; the zero padding relies
    on the runtime's pre-zeroed ExternalOutput buffers
"""

import sys

sys.path.insert(0, "/opt/trn_rl_repo")

import numpy as np

B, E, N, T, F, PAD = 8, 64, 512, 2048, 32768, 65536
L = 16384  # outputs per (event, half) partition
KW = 66  # sample window per 1024-output interp chunk
NCORES = 8

_cache = {}


def _build_winterp() -> np.ndarray:
    """Constant [66, 1024] triangle-filter matrix for 16x linear upsampling.

    Output local index q = 16*m + j consumes window samples s' = m + {0,1,2}:
      j < 8 : taps (15-2j)/32 on m,   (17+2j)/32 on m+1
      j >= 8: taps (47-2j)/32 on m+1, (2j-15)/32 on m+2
    """
    w = np.zeros((KW, 1024), np.float32)
    for q in range(1024):
        m, j = divmod(q, 16)
        if j < 8:
            w[m, q] = (15 - 2 * j) / 32.0
            w[m + 1, q] = (17 + 2 * j) / 32.0
        else:
            w[m + 1, q] = (47 - 2 * j) / 32.0
            w[m + 2, q] = (2 * j - 15) / 32.0
    return w


def _declare_io(nc):
    import concourse.mybir as mybir

    f32 = mybir.dt.float32
    return dict(
        sel_ap=nc.dram_tensor("selections", [E, N], f32, kind="ExternalInput").ap(),
        items_ap=nc.dram_tensor("items", [N, T], f32, kind="ExternalInput").ap(),
        islice_ap=nc.dram_tensor("items_slice", [E, T], f32, kind="ExternalInput").ap(),
        noise_ap=nc.dram_tensor("noise", [E, F], f32, kind="ExternalInput").ap(),
        w_ap=nc.dram_tensor("winterp", [KW, 1024], mybir.dt.bfloat16,
                            kind="ExternalInput").ap(),
        out_ap=nc.dram_tensor("out", [E, PAD], f32, kind="ExternalOutput").ap(),
    )


def _emit(tc, nc, io, variant="v1"):
    import concourse.bass as bass
    import concourse.masks as masks
    import concourse.mybir as mybir
    from concourse import bass_isa

    f32 = mybir.dt.float32
    X = mybir.AxisListType.X
    OP = mybir.AluOpType
    ACT = mybir.ActivationFunctionType

    sel_ap = io["sel_ap"]
    items_ap = io["items_ap"]
    noise_ap = io["noise_ap"]
    w_ap = io["w_ap"]
    out_ap = io["out_ap"]

    # "v1": local full-table scan; "v2": 1/8-slice scan + NRT AllReduce;
    # "floor": min/max skipped entirely (timing probe only).
    # trailing "c": coarse (4 MiB noise / 2 MiB store) DMAs; default fine
    # (1 MiB) — measured faster on HW: per-DMA tile deps gate the epilogue,
    # so finer chunks start stores earlier and shorten the tail.
    fine = not variant.endswith("c")
    if not fine:
        variant = variant[:-1]
    scan = variant == "v1"
    coll = variant == "v2"

    from contextlib import ExitStack

    ctx = ExitStack()
    const = ctx.enter_context(tc.tile_pool(name="const", bufs=1))
    stats = ctx.enter_context(tc.tile_pool(name="stats", bufs=1))
    psA = ctx.enter_context(tc.tile_pool(name="psA", bufs=2, space="PSUM"))
    psI = ctx.enter_context(tc.tile_pool(name="psI", bufs=3, space="PSUM"))
    noisep = ctx.enter_context(tc.tile_pool(name="noisep", bufs=8 if fine else 2))
    outp = ctx.enter_context(tc.tile_pool(name="outp", bufs=4 if fine else 3))
    itemsp = ctx.enter_context(tc.tile_pool(name="itemsp", bufs=1))

    ident = const.tile([128, 128], f32, tag="ident")
    masks.make_identity(nc, ident[:])

    # Sync HWDGE FIFO order = priority order for the critical chain:
    # sel64 (gates argmax -> gather), then the 4 MiB table scan in 8 chunks
    # (gates the min/max -> affine fold; small chunks start the DVE reduces
    # earlier), then the interp matrix, then noise chunks, then stores.
    NSCAN = 8
    SC = (4 * T) // NSCAN  # 1024 columns per scan chunk
    sel64 = const.tile([E, N], f32, tag="sel64")
    nc.sync.dma_start(sel64[:], sel_ap[:])
    if scan:
        itemsF = itemsp.tile([128, 4 * T], f32, tag="itemsF")
        items128 = items_ap.rearrange("(p r) t -> p (r t)", p=128)
        for c in range(NSCAN):
            nc.sync.dma_start(
                itemsF[:, c * SC:(c + 1) * SC], items128[:, c * SC:(c + 1) * SC]
            )
    elif coll:
        isl = itemsp.tile([128, T // 2], f32, tag="islice")
        nc.sync.dma_start(isl[:], io["islice_ap"].rearrange("r (b t2) -> (r b) t2", b=2))
    w_sb = const.tile([KW, 1024], mybir.dt.bfloat16, tag="winterp")
    nc.sync.dma_start(w_sb[:], w_ap[:])

    # noise prefetch: two 4 MiB loads (32 KiB per partition line) — large
    # transfers run near peak HBM BW and cut HWDGE descriptor-gen count
    noise3 = noise_ap.rearrange("e (h x) -> e h x", h=2)
    nts = []
    if fine:
        for cd in range(8):
            nt = noisep.tile([128, 2048], f32, tag="nt")
            nc.sync.dma_start(nt[:], noise3[:, :, 2048 * cd:2048 * (cd + 1)])
            nts.append(nt)
    else:
        for half in range(2):
            nt = noisep.tile([128, 8192], f32, tag="nt")
            nc.sync.dma_start(nt[:], noise3[:, :, 8192 * half:8192 * (half + 1)])
            nts.append(nt)

    # ---- distributed min/max, phase 1 (issued before the argmax chain so
    # the DVE queue starts it the moment islice lands, and the AllReduce
    # launches before the gather occupies the Pool queue) ----
    if coll:
        pkpre = stats.tile([128, 2], f32, tag="pkpre")
        nc.vector.tensor_reduce(pkpre[:, 0:1], isl[:], axis=X, op=OP.max)
        nc.vector.tensor_reduce(pkpre[:, 1:2], isl[:], axis=X, op=OP.min, negate=True)
        pk = stats.tile([128, 2], f32, tag="pk")
        nc.gpsimd.partition_all_reduce(pk[:], pkpre[:], channels=128,
                                       reduce_op=bass_isa.ReduceOp.max)
        dramp = ctx.enter_context(tc.tile_pool(name="dramp", bufs=1, space="DRAM"))
        cin = dramp.tile([1, 2], f32, tag="cin")
        cout = dramp.tile([1, 2], f32, tag="cout")
        nc.scalar.dma_start(cin[:], pk[0:1, 0:2])
        nc.gpsimd.collective_compute(
            "AllReduce", OP.max, replica_groups=[list(range(NCORES))],
            ins=[cin.opt()], outs=[cout.opt()])
        pk2 = stats.tile([1, 2], f32, tag="pk2")
        nc.scalar.dma_start(pk2[:], cout[:])
        pkb = stats.tile([128, 2], f32, tag="pkb")
        nc.gpsimd.partition_broadcast(pkb[:], pk2[:])

    # ---- argmax row indices (top-8 then index-of): gates gather ----
    mx8 = stats.tile([E, 8], f32, tag="mx8")
    nc.vector.max(mx8[:], sel64[:])
    idx8 = stats.tile([E, 8], mybir.dt.uint32, tag="idx8")
    nc.vector.max_index(idx8[:], mx8[:], sel64[:])

    # softmax weight w[e] = 1/sum(exp(x - max)); row max == first of top-8
    negm = stats.tile([E, 1], f32, tag="negm")
    nc.vector.tensor_scalar_mul(negm[:], mx8[:, 0:1], -1.0)
    ex = const.tile([E, N], f32, tag="exp")
    s = stats.tile([E, 1], f32, tag="s")
    nc.scalar.activation(ex[:], sel64[:], ACT.Exp, bias=negm[:], scale=1.0, accum_out=s[:])
    w64 = stats.tile([E, 1], f32, tag="w64")
    nc.vector.reciprocal(w64[:], s[:])

    # ---- gather the argmax rows (one row per event partition) ----
    g_sb = const.tile([E, T + 2], f32, tag="gath")
    nc.gpsimd.indirect_dma_start(
        out=g_sb[:, 1:T + 1], out_offset=None, in_=items_ap[:],
        in_offset=bass.IndirectOffsetOnAxis(ap=idx8[:, 0:1], axis=0))

    # ---- local min/max over the full table -> per-event affine scalars.
    # The 2*NSCAN reduces are issued BEFORE any gather-dependent DVE work so
    # the in-order DVE queue races the chunk DMAs instead of stalling. ----
    if scan:
        pkparts = stats.tile([128, 2 * NSCAN], f32, tag="pkparts")
        for c in range(NSCAN):
            sl = slice(c * SC, (c + 1) * SC)
            nc.vector.tensor_reduce(pkparts[:, c:c + 1], itemsF[:, sl], axis=X, op=OP.max)
            nc.vector.tensor_reduce(pkparts[:, NSCAN + c:NSCAN + c + 1], itemsF[:, sl],
                                    axis=X, op=OP.min, negate=True)
        pkpre = stats.tile([128, 2], f32, tag="pkpre")
        nc.vector.tensor_reduce(pkpre[:, 0:1], pkparts[:, 0:NSCAN], axis=X, op=OP.max)
        nc.vector.tensor_reduce(pkpre[:, 1:2], pkparts[:, NSCAN:2 * NSCAN], axis=X,
                                op=OP.max)
        pk = stats.tile([128, 2], f32, tag="pk")
        nc.gpsimd.partition_all_reduce(pk[:], pkpre[:], channels=128,
                                       reduce_op=bass_isa.ReduceOp.max)
        diffp = stats.tile([E, 1], f32, tag="diffp")
        nc.vector.tensor_scalar(diffp[:], pk[0:E, 0:1], pk[0:E, 1:2], 0.001,
                                op0=OP.add, op1=OP.add)
        inv = stats.tile([E, 1], f32, tag="inv")
        nc.vector.reciprocal(inv[:], diffp[:])
        s0 = stats.tile([E, 1], f32, tag="s0")
        nc.vector.tensor_mul(s0[:], w64[:], inv[:])
        s1 = stats.tile([E, 1], f32, tag="s1")
        nc.vector.tensor_mul(s1[:], s0[:], pk[0:E, 1:2])

    if coll:
        # distributed min/max, phase 2: per-event affine scalars
        diffp = stats.tile([E, 1], f32, tag="diffp")
        nc.vector.tensor_scalar(diffp[:], pkb[0:E, 0:1], pkb[0:E, 1:2], 0.001,
                                op0=OP.add, op1=OP.add)
        inv = stats.tile([E, 1], f32, tag="inv")
        nc.vector.reciprocal(inv[:], diffp[:])
        s0 = stats.tile([E, 1], f32, tag="s0")
        nc.vector.tensor_mul(s0[:], w64[:], inv[:])
        s1 = stats.tile([E, 1], f32, tag="s1")
        nc.vector.tensor_mul(s1[:], s0[:], pkb[0:E, 1:2])

    # edge-clamp halo (DVE, after the scan reduces in queue order)
    nc.vector.tensor_copy(g_sb[:, 0:1], g_sb[:, 1:2])
    nc.vector.tensor_copy(g_sb[:, T + 1:T + 2], g_sb[:, T:T + 1])

    gsc = const.tile([E, T + 2], f32, tag="gsc")
    if scan or coll:
        # gsc = g*(w*inv) + w*(-min)*inv : exact pre-interp normalization
        # fold, on the otherwise-idle ACT engine (Identity shares Exp's
        # act-table set -> no LoadActFuncSet between them)
        nc.scalar.activation(gsc[:], g_sb[:], ACT.Identity, bias=s1[:], scale=s0[:])
    else:
        nc.vector.tensor_scalar(gsc[:], g_sb[:], w64[:], None, op0=OP.mult)

    # ---- window tiles via PE transpose, all up front (PE is idle here and
    # the matmul loop below must not be throttled by transpose/copy work):
    # T[ci][r, 2e+h] = gsc[e, 1024h + 64ci + r], cast to bf16 on the
    # PSUM->SBUF copy (interp taps are k/32 = exact in bf16; sample rounding
    # is 2^-9 relative ~ 0.2% of output scale, far under the gate)
    bf16 = mybir.dt.bfloat16
    T_sb = [const.tile([KW, 128], bf16, tag=f"T{k}", name=f"T{k}") for k in range(16)]
    for ci in range(16):
        for h in range(2):
            ps = psA.tile([KW, E], f32, tag="ps_sel")
            c0 = 1024 * h + 64 * ci
            nc.tensor.transpose(ps[:], gsc[:, c0:c0 + KW], ident[:E, :E])
            nc.scalar.copy(T_sb[ci][0:KW, h:128:2], ps[:])

    # ---- main loop: bf16 interp matmuls (1-pass on PE vs 4-pass fp32) +
    # psum*noise, stream out (2 MiB stores coarse / 1 MiB fine) ----
    outlive3 = out_ap[:, 0:F].rearrange("e (h x) -> e h x", h=2)
    SCW = 2048 if fine else 4096  # store-chunk width per partition
    NSC = 16384 // SCW  # 8 fine / 4 coarse
    for sc in range(NSC):
        ot = outp.tile([128, SCW], f32, tag="ot")
        for q in range(SCW // 1024):
            ci = (SCW // 1024) * sc + q
            ps = psI.tile([128, 1024], f32, tag="ps_interp")
            for nn in range(2):
                sl = slice(nn * 512, (nn + 1) * 512)
                nc.tensor.matmul(ps[:, sl], T_sb[ci][0:KW, :],
                                 w_sb[0:KW, sl], start=True, stop=True)
            if fine:
                nt, nb = nts[ci // 2], (ci % 2) * 1024
            else:
                nt, nb = nts[ci // 8], (ci % 8) * 1024
            nc.vector.tensor_mul(ot[:, q * 1024:(q + 1) * 1024], ps[:],
                                 nt[:, nb:nb + 1024])
        if sc < NSC - 1:
            nc.sync.dma_start(outlive3[:, :, SCW * sc:SCW * (sc + 1)], ot[:])
        else:
            # last chunk: write each half as soon as it is ready to shorten
            # the compute-chain tail
            hw_ = SCW // 2
            for half in range(2):
                nc.sync.dma_start(
                    outlive3[:, :, SCW * sc + hw_ * half:SCW * sc + hw_ * (half + 1)],
                    ot[:, hw_ * half:hw_ * (half + 1)])

    # Zero padding (out[:, F:PAD]) is not written: both run_neff and the
    # PJRT donation path hand the kernel pre-zeroed ExternalOutput buffers.

    ctx.close()


def _bench_io(nc):
    import concourse.mybir as mybir

    f32 = mybir.dt.float32
    return dict(
        sel_ap=nc.dram_tensor("selections", [E, N], f32).ap(),
        items_ap=nc.dram_tensor("items", [N, T], f32).ap(),
        islice_ap=nc.dram_tensor("items_slice", [E, T], f32).ap(),
        noise_ap=nc.dram_tensor("noise", [E, F], f32).ap(),
        w_ap=nc.dram_tensor("winterp", [KW, 1024], mybir.dt.bfloat16).ap(),
        out_ap=nc.dram_tensor("out", [E, PAD], f32).ap(),
    )


def _program_bench(reps: int = 1, barrier: bool = False, variant: str = "v2"):
    """Timing-only variant: all real I/O lives in Internal DRAM (no host
    transfer), one dummy external in/out so the PJRT path has operands.
    barrier=True serializes reps (per-rep = single-shot latency)."""
    key = ("bench", reps, barrier, variant)
    if key in _cache:
        return _cache[key]
    import concourse.bacc as bacc
    import concourse.mybir as mybir
    import concourse.tile as tile

    f32 = mybir.dt.float32
    nc = bacc.Bacc("TRN2", target_bir_lowering=False, debug=False,
                   num_devices=NCORES)
    dummy_in = nc.dram_tensor("bench_in", [1, 128], f32, kind="ExternalInput").ap()
    dummy_out = nc.dram_tensor("bench_out", [1, 128], f32, kind="ExternalOutput").ap()
    io = _bench_io(nc)
    with tile.TileContext(nc) as tc:
        for i in range(reps):
            if barrier and i:
                tc.strict_bb_all_engine_barrier()
            _emit(tc, nc, io, variant=variant)
        with tc.tile_pool(name="dummyp", bufs=1) as dp:
            t = dp.tile([1, 128], f32)
            nc.sync.dma_start(t[:], dummy_in[:])
            nc.sync.dma_start(dummy_out[:], t[:])
    nc.compile()
    _cache[key] = nc
    return nc


def _program_bench_loop(n_iters: int, variant: str = "v1"):
    """Timing twin with a HARDWARE loop around the kernel body: one NEFF
    dispatch runs the body n_iters times (For_i resets semaphores with an
    all-engine barrier each iteration => per-iteration single-shot latency).
    Device time scales with n_iters at zero extra dispatch overhead, which
    beats the RPC jitter of the axon tunnel.  Valid for this kernel because
    it contains no collectives."""
    key = ("benchloop", n_iters, variant)
    if key in _cache:
        return _cache[key]
    import concourse.bacc as bacc
    import concourse.mybir as mybir
    import concourse.tile as tile

    f32 = mybir.dt.float32
    nc = bacc.Bacc("TRN2", target_bir_lowering=False, debug=False,
                   num_devices=NCORES)
    dummy_in = nc.dram_tensor("bench_in", [1, 128], f32, kind="ExternalInput").ap()
    dummy_out = nc.dram_tensor("bench_out", [1, 128], f32, kind="ExternalOutput").ap()
    io = _bench_io(nc)
    with tile.TileContext(nc) as tc:
        with tc.For_i(0, n_iters):
            _emit(tc, nc, io, variant=variant)
        with tc.tile_pool(name="dummyp", bufs=1) as dp:
            t = dp.tile([1, 128], f32)
            nc.sync.dma_start(t[:], dummy_in[:])
            nc.sync.dma_start(dummy_out[:], t[:])
    nc.compile()
    _cache[key] = nc
    return nc


def _program(reps: int = 1):
    key = ("nc", reps)
    if key in _cache:
        return _cache[key]
    import concourse.bacc as bacc
    import concourse.tile as tile

    nc = bacc.Bacc("TRN2", target_bir_lowering=False, debug=False,
                   num_devices=NCORES)
    io = _declare_io(nc)
    with tile.TileContext(nc) as tc:
        for _ in range(reps):
            _emit(tc, nc, io, variant="v2")
    nc.compile()
    _cache[key] = nc
    return nc


def kernel(selections: np.ndarray, items: np.ndarray, noise: np.ndarray) -> np.ndarray:
    from concourse.bass_utils import run_bass_kernel_spmd

    import ml_dtypes

    nc = _program()
    winterp = _build_winterp().astype(ml_dtypes.bfloat16)  # taps k/32: exact
    sel = np.ascontiguousarray(np.asarray(selections, np.float32))
    it = np.ascontiguousarray(np.asarray(items, np.float32))
    nz = np.ascontiguousarray(np.asarray(noise, np.float32))
    in_maps = [
        {"selections": sel[b], "items": it,
         "items_slice": it[E * b:E * (b + 1)],
         "noise": nz[b], "winterp": winterp}
        for b in range(NCORES)
    ]
    last_err = None
    for attempt in range(3):
        try:
            res = run_bass_kernel_spmd(nc, in_maps, list(range(NCORES)))
            break
        except Exception as e:  # transient NRT device wedge: retry
            last_err = e
            import time as _time

            _time.sleep(2.0 * (attempt + 1))
    else:
        # an in-process retry does not clear an NRT device wedge, but a
        # fresh process does — run once in a subprocess as a last resort
        import os

        if os.environ.get("BASS_KERNEL_NO_SUBPROC"):
            raise last_err
        return _run_in_subprocess(sel, it, nz)
    return np.stack([res.results[b]["out"] for b in range(NCORES)]).astype(np.float32)


def _run_in_subprocess(sel, it, nz):
    import os
    import subprocess
    import tempfile

    d = tempfile.mkdtemp()
    np.save(os.path.join(d, "sel.npy"), sel)
    np.save(os.path.join(d, "items.npy"), it)
    np.save(os.path.join(d, "noise.npy"), nz)
    mydir = os.path.dirname(os.path.abspath(__file__))
    driver = os.path.join(d, "drv.py")
    with open(driver, "w") as f:
        f.write(
            "import sys, numpy as np\n"
            f"sys.path.insert(0, {mydir!r})\n"
            "import kernel\n"
            f"d = {d!r}\n"
            "out = kernel.kernel(selections=np.load(d + '/sel.npy'),\n"
            "                    items=np.load(d + '/items.npy'),\n"
            "                    noise=np.load(d + '/noise.npy'))\n"
            "np.save(d + '/out.npy', out)\n")
    env = dict(os.environ)
    env["BASS_KERNEL_NO_SUBPROC"] = "1"
    subprocess.run([sys.executable, driver], check=True, env=env)
    return np.load(os.path.join(d, "out.npy"))
